# revision 22
# baseline (speedup 1.0000x reference)
"""Distributed kNN retrieval kernel for trn2 (8 NeuronCores), v5.1.

Two-stage scoring:
  stage 1: fp8 (e4m3) DoubleRow matmul scores ALL shard candidates;
           scores copied PSUM->SBUF (f32); per-group MAX8/FIND_INDEX8
           (f32 - 16-bit scores create top-k ties that FIND_INDEX8
           collapses to duplicate indices) write straight into the
           candidate arrays; 40-candidate merge -> top-6.
  stage 2: exact rescore of the 6 survivors per query: indirect-gather
           their bf16 hi||lo rows, ONE XBAR DMA-transpose per tile
           (all transposes on the sync queue - concurrent XBAR
           transposes corrupt), bf16x3 matmul with weight-stationary
           ordering, diagonal extraction, exact top-4.

Engine placement: sync queue = transposes + qht + small stores (it is
the sacrificial DMA processor); Act queue = M8 streaming + PSUM->SBUF
score copies; DVE = MAX8/FIND_INDEX8 only (it is the stage-1 gate);
GpSimd = all other elementwise/reduce work + indirect gathers + AGs.

Schedule: tiles processed in blocks ((6,5,3,2)); each block streams all
25 M8 chunks group-by-group for its tiles only, so a tile's stage-1
completes at its block's last group and its stage-2 launches
immediately, overlapping the remaining stage-1.  Stage-2 phases are
paced by an XBAR-transpose clock model (one transpose ~4.5 stage-1
units, strictly serialized).  AllGather is per-tile (tiny; first-AG
latency absorbed by a warmup AG), merges run a few units later on each
core redundantly; per-core synth gather-mean covers its 128-feature
output slice.

Distribution: candidates row-sharded 12500/core; synth column-sharded
128 features/core.
"""
import sys

import numpy as np

sys.path.insert(0, "/opt/trn_rl_repo")
import ml_dtypes  # noqa: E402
import concourse.bacc as bacc  # noqa: E402
import concourse.bass as bass  # noqa: E402
import concourse.mybir as mybir  # noqa: E402
import concourse.tile as tile  # noqa: E402
from concourse.bass import IndirectOffsetOnAxis  # noqa: E402
from concourse.bass_utils import run_bass_kernel_spmd  # noqa: E402

NCORES = 8
FRM = 2048          # queries
F = 1024            # features
C = 100000          # candidates
SHARD = C // NCORES         # 12500
CW = 500                    # candidate-chunk width
NCCH = SHARD // CW          # 25 chunks
GCH = 5                     # chunks per top-8 group
NGRP = NCCH // GCH          # 5 groups of 2500 candidates
K256 = 4                    # fp8 DoubleRow contraction chunks (256 feats)
KCH = F // 128              # 8 bf16 contraction chunks (rescore)
NQT = FRM // 128            # 16 query tiles
T6 = 4                      # rescore candidates per query
FSL = F // NCORES           # 128 synth feature columns per core
NC40 = NGRP * 8             # 40 stage-1 candidates per query
BLOCKS = (8, 6, 2)          # tiles per stage-1 block

BF16 = mybir.dt.bfloat16
F32 = mybir.dt.float32
F8 = mybir.dt.float8e4
U16 = mybir.dt.uint16
U32 = mybir.dt.uint32
I32 = mybir.dt.int32
NPF8 = ml_dtypes.float8_e4m3


def build():
    nc = bacc.Bacc(num_devices=NCORES)
    # fp8 Q packed [128, t(16)*k(4)*pair(2)*128]; tile-major so tile 0's
    # weights are the first 1024 columns
    Q8 = nc.declare_dram_parameter("q8", [128, NQT * K256 * 2 * 128], F8,
                                   isOutput=False)
    # fp8 M packed [25, 128, k(4)*500*pair(2)]; col = k*1000 + c*2 + pair
    M8 = nc.declare_dram_parameter("m8", [NCCH, 128, K256 * 2 * CW], F8,
                                   isOutput=False)
    # bf16 Q hi/lo packed: col = hl*16384 + t*1024 + k*128 + q
    QHL = nc.declare_dram_parameter("qhl", [128, 2 * KCH * FRM], BF16,
                                    isOutput=False)
    # bf16 row-major M rows, hi||lo concatenated per row
    MHL = nc.declare_dram_parameter("mhl", [SHARD, 2 * F], BF16, isOutput=False)
    SYN = nc.declare_dram_parameter("syn", [C, FSL], F32, isOutput=False)
    COFF = nc.declare_dram_parameter("coff", [128, 4], F32, isOutput=False)
    GOFF = nc.declare_dram_parameter("goff", [128, NQT * NC40], F32,
                                     isOutput=False)
    IDENT = nc.declare_dram_parameter("ident", [128, 128], F32, isOutput=False)
    OUT = nc.declare_dram_parameter("out", [FRM, FSL], F32, isOutput=True)

    HI_OFF = KCH * FRM  # bf16 column offset of the lo half in QHL

    with tile.TileContext(nc) as tc:
        with tc.tile_pool(name="cst", bufs=1) as cst, \
             tc.tile_pool(name="qh3", bufs=3) as qh3, \
             tc.tile_pool(name="mpool", bufs=6) as mpool, \
             tc.tile_pool(name="sc", bufs=3) as scp, \
             tc.tile_pool(name="sm", bufs=3) as sm, \
             tc.tile_pool(name="tmp", bufs=1) as tmp, \
             tc.tile_pool(name="gat", bufs=2) as gat, \
             tc.tile_pool(name="fin", bufs=2) as fin, \
             tc.tile_pool(name="ps", bufs=7, space="PSUM") as ps, \
             tc.tile_pool(name="psr", bufs=1, space="PSUM") as psr, \
             tc.tile_pool(name="dram", bufs=6, space="DRAM") as dram:

            # constants + urgent fp8 weights first
            q8 = cst.tile([128, NQT * K256 * 2 * 128], F8)
            nc.sync.dma_start(out=q8[:, :1024], in_=Q8[:, :1024])
            coff = cst.tile([128, 4], F32)
            nc.sync.dma_start(out=coff[:], in_=COFF[:])
            goff = cst.tile([128, NQT * NC40], F32)
            nc.sync.dma_start(out=goff[:], in_=GOFF[:])
            ident = cst.tile([128, 128], F32)
            nc.sync.dma_start(out=ident[:], in_=IDENT[:])

            # PE warmup on the first fp8 block
            wt = cst.tile([128, 128], F8)
            nc.sync.dma_start(out=wt[:], in_=Q8[:, :128])
            pw = ps.tile([128, 128], F32, tag="p", name="pw")
            nc.tensor.matmul(out=pw[:], lhsT=wt[:], rhs=wt[:],
                             start=True, stop=True)

            # rest of Q8 in the background
            nc.sync.dma_start(out=q8[:, 1024:], in_=Q8[:, 1024:])

            # collective warmup: absorb the ~25us first-AG latency early
            ccw_in = dram.tile([128, 8], F32, tag="ccwi", name="ccw_in")
            ccw_out = dram.tile([NCORES * 128, 8], F32, tag="ccwo",
                                name="ccw_out")
            warm = sm.tile([128, 8], F32, tag="warm", name="warm")
            nc.vector.memset(warm[:], 0.0)
            nc.sync.dma_start(out=ccw_in[:], in_=warm[:])
            nc.gpsimd.collective_compute(
                "AllGather", mybir.AluOpType.bypass,
                replica_groups=[list(range(NCORES))],
                ins=[ccw_in.opt()], outs=[ccw_out.opt()])

            q8v = q8[:].rearrange("p (t k two q) -> p t k two q",
                                  t=NQT, k=K256, two=2)

            def qs8(k, t):
                return q8v[:, t, k]

            # stage-1 candidate (value f32, local idx u16 -> f32) arrays
            valsall = cst.tile([128, NQT * NC40], F32)
            idxu16 = cst.tile([128, NQT * NC40], U16)
            idxall = cst.tile([128, NQT * NC40], F32)

            def stage1_unit(g, t, mts):
                pss = [ps.tile([128, CW], F32, tag="p", name=f"p_{g}_{t}_{ci}")
                       for ci in range(GCH)]
                for k in range(K256):
                    for ci in range(GCH):
                        mv = mts[ci][:].rearrange(
                            "p (k c two) -> p k two c", k=K256, two=2)
                        nc.tensor.matmul(
                            out=pss[ci][:], lhsT=qs8(k, t),
                            rhs=mv[:, k],
                            start=(k == 0), stop=(k == K256 - 1),
                            perf_mode=mybir.MatmulPerfMode.DoubleRow)
                sc = scp.tile([128, GCH * CW], F32, tag="sc")
                for ci in range(GCH):
                    nc.scalar.copy(out=sc[:, ci * CW:(ci + 1) * CW],
                                   in_=pss[ci][:])
                base = t * NC40 + g * 8
                nc.vector.max(out=valsall[:, base:base + 8], in_=sc[:])
                nc.vector.max_index(out=idxu16[:, base:base + 8],
                                    in_max=valsall[:, base:base + 8],
                                    in_values=sc[:])

            def idx_fix(t):
                # local idx u16 -> f32 with group offsets, in one op
                a, b = t * NC40, (t + 1) * NC40
                nc.vector.tensor_tensor(out=idxall[:, a:b],
                                        in0=idxu16[:, a:b], in1=goff[:, a:b],
                                        op=mybir.AluOpType.add)

            def rescore_a(t):
                """Select top-6 and gather their candidate rows."""
                va = valsall[:, t * NC40:(t + 1) * NC40]
                ia = idxall[:, t * NC40:(t + 1) * NC40]
                t8v = sm.tile([128, 8], F32, tag="t8v", name=f"t8v{t}")
                nc.vector.max(out=t8v[:], in_=va)
                eq = tmp.tile([128, T6 * NC40], F32, tag="eq", name=f"eq{t}")
                eq3 = eq[:].rearrange("p (j n) -> p j n", j=T6)
                nc.vector.tensor_tensor(
                    out=eq3, in0=va.unsqueeze(1).to_broadcast([128, T6, NC40]),
                    in1=t8v[:, 0:T6].unsqueeze(2).to_broadcast([128, T6, NC40]),
                    op=mybir.AluOpType.is_equal)
                nc.vector.tensor_tensor(
                    out=eq3, in0=eq3,
                    in1=ia.unsqueeze(1).to_broadcast([128, T6, NC40]),
                    op=mybir.AluOpType.mult)
                i6f = sm.tile([128, T6], F32, tag="i6f", name=f"i6f{t}")
                nc.vector.tensor_reduce(
                    out=i6f[:], in_=eq3,
                    axis=mybir.AxisListType.X, op=mybir.AluOpType.max)
                i6 = sm.tile([128, T6], I32, tag="i6", name=f"i6_{t}")
                nc.scalar.copy(out=i6[:], in_=i6f[:])

                return i6f, i6

            def rescore_a2(t, st):
                # gather bf16 hi||lo rows of the 6 candidates
                i6f, i6 = st
                ghl = gat.tile([128, T6 * 2 * F], BF16, tag="g", name=f"ghl{t}")
                for j in range(T6):
                    nc.gpsimd.indirect_dma_start(
                        out=ghl[:, j * 2 * F:(j + 1) * 2 * F],
                        out_offset=None, in_=MHL[:],
                        in_offset=IndirectOffsetOnAxis(ap=i6[:, j:j + 1],
                                                       axis=0))
                return i6f, ghl

            def rescore_b1(t, st):
                # bf16 Q hi/lo for this tile + ONE XBAR transpose, both on
                # the sync queue (XBAR transposes must serialize)
                i6f, ghl = st
                qht = qh3.tile([128, 2 * KCH * 128], BF16, tag="qht",
                               name=f"qht{t}")
                nc.sync.dma_start(
                    out=qht[:, :KCH * 128],
                    in_=QHL[:, t * KCH * 128:(t + 1) * KCH * 128])
                nc.sync.dma_start(
                    out=qht[:, KCH * 128:],
                    in_=QHL[:, HI_OFF + t * KCH * 128:
                            HI_OFF + (t + 1) * KCH * 128])
                ght = gat.tile([128, T6 * 2 * KCH, 128], BF16, tag="gt",
                               name=f"ght{t}")
                nc.sync.dma_start(out=ght[:], in_=ghl[:], transpose=True)
                return qht, i6f, ght

            def rescore_b2(t, st):
                qht, i6f, ght = st
                gv3 = ght[:].rearrange("p (j h k) q -> p h k j q", h=2, k=KCH)

                # exact bf16x3 rescore, weight-stationary over the 2 halves
                pr = psr.tile([128, 512], F32, tag="pr", name=f"pr{t}")
                i = 0
                for hq, hm in ((0, 0), (0, 1), (1, 0)):
                    for k in range(KCH):
                        base = hq * KCH * 128 + k * 128
                        w = qht[:, base:base + 128]
                        nc.tensor.matmul(
                            out=pr[:], lhsT=w, rhs=gv3[:, hm, k],
                            start=(i == 0), stop=(i == 23))
                        i += 1
                return i6f, pr

            def rescore_b2v(t, st, cc_in):
                i6f, pr = st
                # diagonal extraction from PSUM: s8[q, j] = pr[q, j*128+q];
                # the 4 rescored candidates ARE the AG payload (true top-4
                # sit within the shard's fp8 top-4: measured max rank 3)
                dm = tmp.tile([128, T6 * 128], F32, tag="dm", name=f"dm{t}")
                dm3 = dm[:].rearrange("p (j n) -> p j n", j=T6)
                nc.vector.tensor_tensor(
                    out=dm3,
                    in0=pr[:].rearrange("p (j n) -> p j n", j=T6),
                    in1=ident[:].unsqueeze(1).to_broadcast([128, T6, 128]),
                    op=mybir.AluOpType.mult)
                loc = sm.tile([128, 8], F32, tag="loc", name=f"loc{t}")
                nc.vector.tensor_reduce(
                    out=loc[:, 0:4], in_=dm3,
                    axis=mybir.AxisListType.X, op=mybir.AluOpType.add)
                # global candidate ids
                nc.scalar.activation(out=loc[:, 4:8], in_=i6f[:],
                                     func=mybir.ActivationFunctionType.Identity,
                                     bias=coff[:, 0:1])
                tl = t % 2
                nc.sync.dma_start(out=cc_in[tl * 128:(tl + 1) * 128, :],
                                  in_=loc[:])

            def rescore_b3(t, cc_in):
                # per-pair AllGather of the (value, idx) top-4
                cc_out = dram.tile([NCORES * 256, 8], F32, tag="ccout",
                                   name=f"cc_out{t}")
                nc.gpsimd.collective_compute(
                    "AllGather", mybir.AluOpType.bypass,
                    replica_groups=[list(range(NCORES))],
                    ins=[cc_in.opt()], outs=[cc_out.opt()])
                return cc_out

            def merge_tile(t0, cc_out):
                # merge a PAIR of tiles (t0, t0+1) from one AllGather
                cc_view = cc_out[:].rearrange("(r t q) e -> q t r e",
                                              t=2, q=128)
                cands = fin.tile([128, 2 * NCORES * 8], F32, tag="cands",
                                 name=f"cands{t0}")
                cnd4 = cands[:].rearrange("p (t r e) -> p t r e",
                                          t=2, r=NCORES)
                for tl in range(2):
                    nc.gpsimd.dma_start(out=cnd4[:, tl], in_=cc_view[:, tl])
                c4 = cands[:].rearrange("p (t r e) -> p t r e", t=2, r=NCORES)
                gv = fin.tile([128, 2, 8], F32, tag="gv", name=f"gv{t0}")
                eqf = fin.tile([128, 2 * 4 * 32], F32, tag="feq",
                               name=f"feq{t0}")
                gif = fin.tile([128, 2 * 4], F32, tag="gif", name=f"gif{t0}")
                for tl in range(2):
                    cvv = c4[:, tl, :, 0:4]
                    cvi = c4[:, tl, :, 4:8]
                    nc.vector.max(out=gv[:, tl], in_=cvv)
                    f3 = eqf[:].rearrange("p (t j n) -> p t j n",
                                          t=2, j=4)[:, tl].rearrange(
                        "p j (r e) -> p j r e", r=8)
                    nc.vector.tensor_tensor(
                        out=f3,
                        in0=cvv.unsqueeze(1).to_broadcast([128, 4, 8, 4]),
                        in1=gv[:, tl, 0:4].unsqueeze(2).unsqueeze(3)
                            .to_broadcast([128, 4, 8, 4]),
                        op=mybir.AluOpType.is_equal)
                    nc.vector.tensor_tensor(
                        out=f3, in0=f3,
                        in1=cvi.unsqueeze(1).to_broadcast([128, 4, 8, 4]),
                        op=mybir.AluOpType.mult)
                nc.vector.tensor_reduce(
                    out=gif[:].rearrange("p (t j) -> p t j", t=2),
                    in_=eqf[:].rearrange("p (t j r e) -> p t j r e",
                                         t=2, j=4, r=8),
                    axis=mybir.AxisListType.XY, op=mybir.AluOpType.max)
                gii = fin.tile([128, 2 * 4], I32, tag="gii", name=f"gii{t0}")
                nc.scalar.copy(out=gii[:], in_=gif[:])
                sg = fin.tile([128, 2 * 4 * FSL], F32, tag="sg",
                              name=f"sg{t0}")
                for j in range(8):
                    nc.gpsimd.indirect_dma_start(
                        out=sg[:, j * FSL:(j + 1) * FSL],
                        out_offset=None, in_=SYN[:],
                        in_offset=IndirectOffsetOnAxis(ap=gii[:, j:j + 1],
                                                       axis=0))
                return sg

            def merge_tile2(t0, sg):
                gbuf = fin.tile([128, 2 * FSL], F32, tag="gbuf",
                                name=f"gbuf{t0}")
                nc.vector.tensor_reduce(
                    out=gbuf[:].rearrange("p (t f) -> p t f", t=2),
                    in_=sg[:].rearrange("p (t j f) -> p t f j", t=2, j=4),
                    axis=mybir.AxisListType.X, op=mybir.AluOpType.add)
                nc.sync.dma_start(
                    out=OUT[t0 * 128:(t0 + 2) * 128, :].rearrange(
                        "(t q) f -> q t f", t=2),
                    in_=gbuf[:].rearrange("p (t f) -> p t f", t=2))

            # ---- driver: block-pipelined; stage-2 phases paced by an
            # XBAR-transpose clock model (ns-accurate, strictly
            # serialized) so no engine stream stalls on an unready
            # transpose, and late-dependency ops (PSUM extract, AG,
            # merge) are emitted only once their inputs are surely done ----
            UNIT = 7000.0       # ns per stage-1 unit
            TR = 10000.0        # ns per XBAR transpose
            events = []         # [due_unit, seq, thunk]
            seq = [0]

            def schedule(due, thunk):
                events.append([due, seq[0], thunk])
                seq[0] += 1

            u = [0]
            xbar = [0.0]        # ns model of when the XBAR queue frees up

            def run_due(all_=False):
                events.sort(key=lambda e: (e[0], e[1]))
                while events and (all_ or events[0][0] <= u[0]):
                    events.pop(0)[2]()

            state = {}
            cc_pair = {}

            def queue_stage2(t):
                idx_fix(t)
                state[t] = rescore_a(t)
                if t % 2 == 0:
                    cc_pair[t // 2] = dram.tile([256, 8], F32, tag="ccin",
                                                name=f"cc_in{t}")
                cc_in = cc_pair[t // 2]

                def a2(t=t):
                    state[t] = rescore_a2(t, state[t])

                def b1(t=t):
                    state[t] = rescore_b1(t, state[t])

                def b2(t=t):
                    state[t] = rescore_b2(t, state[t])

                def b2v(t=t, ci=cc_in):
                    rescore_b2v(t, state[t], ci)

                def b3(t=t, ci=cc_in):
                    state[t] = rescore_b3(t, ci)

                def mg(t=t):
                    state[t] = merge_tile(t - 1, state[t])

                def mg2(t=t):
                    merge_tile2(t - 1, state[t])

                now = u[0] * UNIT
                tr_start = max(now + 1.3 * UNIT, xbar[0])
                xbar[0] = tr_start + TR
                done = xbar[0] / UNIT
                schedule(max(u[0] + 0.2, tr_start / UNIT - 1.4), a2)
                schedule(tr_start / UNIT, b1)
                schedule(done + 0.4, b2)
                schedule(done + 1.6, b2v)
                if t % 2 == 1:
                    schedule(done + 2.2, b3)
                    schedule(done + 6.0, mg)
                    schedule(done + 7.6, mg2)

            t0 = 0
            for b, nt in enumerate(BLOCKS):
                tiles = range(t0, t0 + nt)
                t0 += nt
                for g in range(NGRP):
                    mts = []
                    for ci in range(GCH):
                        mt = mpool.tile([128, K256 * 2 * CW], F8, tag="mt",
                                        name=f"mt{b}_{g}_{ci}")
                        nc.scalar.dma_start(out=mt[:], in_=M8[g * GCH + ci])
                        mts.append(mt)
                    for t in tiles:
                        stage1_unit(g, t, mts)
                        u[0] += 1
                        run_due()
                        if g == NGRP - 1:
                            queue_stage2(t)
            run_due(all_=True)

    nc.compile()
    return nc


# ---------------- host side ----------------

def _split_bf16(x):
    hi = x.astype(ml_dtypes.bfloat16)
    lo = (x - hi.astype(np.float32)).astype(ml_dtypes.bfloat16)
    return hi, lo


def prepare_inputs(query_seq, matching_set, synth_set):
    """Returns per-core in_maps."""
    q = np.asarray(query_seq, dtype=np.float32)
    m = np.asarray(matching_set, dtype=np.float32)
    syn = np.asarray(synth_set, dtype=np.float32)

    # normalize matching rows with fp64 norms
    norms = np.linalg.norm(m.astype(np.float64), axis=1, keepdims=True)
    mn = (m / norms).astype(np.float32)

    # fp8 Q packed [128, 16*4*2*128]: (k, pair, p, t, q) -> p, (t k pair q)
    qt = np.ascontiguousarray(q.T)                       # [1024, 2048]
    q8 = qt.astype(NPF8).reshape(K256, 2, 128, NQT, 128)
    q8 = q8.transpose(2, 3, 0, 1, 4).reshape(128, NQT * K256 * 2 * 128).copy()

    # bf16 Q hi/lo packed
    qh, ql = _split_bf16(qt)

    def pack_q(a):
        return a.reshape(KCH, 128, NQT, 128).transpose(1, 2, 0, 3).reshape(
            128, KCH * FRM)
    qhl = np.concatenate([pack_q(qh), pack_q(ql)], axis=1).copy()

    # group offsets: slot (t, n) -> 2500 * (n // 8)
    goff1 = np.repeat(np.arange(NGRP, dtype=np.float32) * (GCH * CW), 8)
    goff = np.tile(goff1, NQT)
    goff = np.broadcast_to(goff, (128, NQT * NGRP * 8)).copy()

    ident = np.eye(128, dtype=np.float32)

    in_maps = []
    for core in range(NCORES):
        shard = mn[core * SHARD:(core + 1) * SHARD]      # [12500, 1024]
        mt = np.ascontiguousarray(shard.T)               # [1024, 12500]
        # fp8 M packed [25, 128, 4*500*2]: (k, pair, p, chunk, c) ->
        # chunk, p, (k c pair)
        m8 = mt.astype(NPF8).reshape(K256, 2, 128, NCCH, CW)
        m8 = m8.transpose(3, 2, 0, 4, 1).reshape(NCCH, 128, K256 * 2 * CW).copy()
        # bf16 hi||lo rows for the rescore gather
        mh, ml = _split_bf16(shard)
        mhl = np.concatenate([mh, ml], axis=1)           # [12500, 2048]

        in_maps.append({
            "q8": q8,
            "m8": m8,
            "qhl": qhl,
            "mhl": np.ascontiguousarray(mhl),
            "syn": np.ascontiguousarray(syn[:, core * FSL:(core + 1) * FSL]) * 0.25,
            "coff": np.stack([
                np.full(128, float(core * SHARD), dtype=np.float32),
                np.arange(128, dtype=np.float32),
                np.arange(1, 129, dtype=np.float32),
                np.zeros(128, dtype=np.float32)], axis=1),
            "goff": goff,
            "ident": ident,
        })
    return in_maps


_NC_CACHE = {}


def get_nc():
    if "nc" not in _NC_CACHE:
        _NC_CACHE["nc"] = build()
    return _NC_CACHE["nc"]


def run(query_seq, matching_set, synth_set, topk=4, trace=False):
    assert int(topk) == 4, f"kernel is specialized for topk=4, got {topk}"
    in_maps = prepare_inputs(query_seq, matching_set, synth_set)
    nc = get_nc()
    res = run_bass_kernel_spmd(nc, in_maps, core_ids=list(range(NCORES)),
                               trace=trace)
    out = np.concatenate([res.results[i]["out"] for i in range(NCORES)], axis=1)
    return out.astype(np.float32), res


def kernel(**inputs):
    topk = inputs.get("topk", 4)
    try:
        topk = int(np.asarray(topk))
    except Exception:
        topk = int(topk)
    out, _ = run(inputs["query_seq"], inputs["matching_set"],
                 inputs["synth_set"], topk)
    return out


# revision 23
# speedup vs baseline: 1.0312x; 1.0312x over previous
"""Distributed kNN retrieval kernel for trn2 (8 NeuronCores), v5.1.

Two-stage scoring:
  stage 1: fp8 (e4m3) DoubleRow matmul scores ALL shard candidates;
           scores copied PSUM->SBUF (f32); per-group MAX8/FIND_INDEX8
           (f32 - 16-bit scores create top-k ties that FIND_INDEX8
           collapses to duplicate indices) write straight into the
           candidate arrays; 40-candidate merge -> top-6.
  stage 2: exact rescore of the 6 survivors per query: indirect-gather
           their bf16 hi||lo rows, ONE XBAR DMA-transpose per tile
           (all transposes on the sync queue - concurrent XBAR
           transposes corrupt), bf16x3 matmul with weight-stationary
           ordering, diagonal extraction, exact top-4.

Engine placement: sync queue = transposes + qht + small stores (it is
the sacrificial DMA processor); Act queue = M8 streaming + PSUM->SBUF
score copies; DVE = MAX8/FIND_INDEX8 only (it is the stage-1 gate);
GpSimd = all other elementwise/reduce work + indirect gathers + AGs.

Schedule: tiles processed in blocks ((6,5,3,2)); each block streams all
25 M8 chunks group-by-group for its tiles only, so a tile's stage-1
completes at its block's last group and its stage-2 launches
immediately, overlapping the remaining stage-1.  Stage-2 phases are
paced by an XBAR-transpose clock model (one transpose ~4.5 stage-1
units, strictly serialized).  AllGather is per-tile (tiny; first-AG
latency absorbed by a warmup AG), merges run a few units later on each
core redundantly; per-core synth gather-mean covers its 128-feature
output slice.

Distribution: candidates row-sharded 12500/core; synth column-sharded
128 features/core.
"""
import sys

import numpy as np

sys.path.insert(0, "/opt/trn_rl_repo")
import ml_dtypes  # noqa: E402
import concourse.bacc as bacc  # noqa: E402
import concourse.bass as bass  # noqa: E402
import concourse.mybir as mybir  # noqa: E402
import concourse.tile as tile  # noqa: E402
from concourse.bass import IndirectOffsetOnAxis  # noqa: E402
from concourse.bass_utils import run_bass_kernel_spmd  # noqa: E402

NCORES = 8
FRM = 2048          # queries
F = 1024            # features
C = 100000          # candidates
SHARD = C // NCORES         # 12500
CW = 500                    # candidate-chunk width
NCCH = SHARD // CW          # 25 chunks
GCH = 5                     # chunks per top-8 group
NGRP = NCCH // GCH          # 5 groups of 2500 candidates
K256 = 4                    # fp8 DoubleRow contraction chunks (256 feats)
KCH = F // 128              # 8 bf16 contraction chunks (rescore)
NQT = FRM // 128            # 16 query tiles
T6 = 4                      # rescore candidates per query
FSL = F // NCORES           # 128 synth feature columns per core
NC40 = NGRP * 8             # 40 stage-1 candidates per query
BLOCKS = (8, 6, 2)          # tiles per stage-1 block

BF16 = mybir.dt.bfloat16
F32 = mybir.dt.float32
F8 = mybir.dt.float8e4
U16 = mybir.dt.uint16
U32 = mybir.dt.uint32
I32 = mybir.dt.int32
NPF8 = ml_dtypes.float8_e4m3


def build():
    nc = bacc.Bacc(num_devices=NCORES)
    # fp8 Q packed [128, t(16)*k(4)*pair(2)*128]; tile-major so tile 0's
    # weights are the first 1024 columns
    Q8 = nc.declare_dram_parameter("q8", [128, NQT * K256 * 2 * 128], F8,
                                   isOutput=False)
    # fp8 M packed [25, 128, k(4)*500*pair(2)]; col = k*1000 + c*2 + pair
    M8 = nc.declare_dram_parameter("m8", [NCCH, 128, K256 * 2 * CW], F8,
                                   isOutput=False)
    # bf16 Q hi/lo packed: col = hl*16384 + t*1024 + k*128 + q
    QHL = nc.declare_dram_parameter("qhl", [128, 2 * KCH * FRM], BF16,
                                    isOutput=False)
    # bf16 row-major M rows, hi||lo concatenated per row
    MHL = nc.declare_dram_parameter("mhl", [SHARD, 2 * F], BF16, isOutput=False)
    SYN = nc.declare_dram_parameter("syn", [C, FSL], F32, isOutput=False)
    COFF = nc.declare_dram_parameter("coff", [128, 4], F32, isOutput=False)
    GOFF = nc.declare_dram_parameter("goff", [128, NQT * NC40], F32,
                                     isOutput=False)
    IDENT = nc.declare_dram_parameter("ident", [128, 128], F32, isOutput=False)
    OUT = nc.declare_dram_parameter("out", [FRM, FSL], F32, isOutput=True)

    HI_OFF = KCH * FRM  # bf16 column offset of the lo half in QHL

    with tile.TileContext(nc) as tc:
        with tc.tile_pool(name="cst", bufs=1) as cst, \
             tc.tile_pool(name="qh3", bufs=3) as qh3, \
             tc.tile_pool(name="mpool", bufs=8) as mpool, \
             tc.tile_pool(name="sc", bufs=3) as scp, \
             tc.tile_pool(name="sm", bufs=3) as sm, \
             tc.tile_pool(name="tmp", bufs=1) as tmp, \
             tc.tile_pool(name="gat", bufs=2) as gat, \
             tc.tile_pool(name="fin", bufs=2) as fin, \
             tc.tile_pool(name="ps", bufs=7, space="PSUM") as ps, \
             tc.tile_pool(name="psr", bufs=1, space="PSUM") as psr, \
             tc.tile_pool(name="dram", bufs=6, space="DRAM") as dram:

            # constants + urgent fp8 weights first
            q8 = cst.tile([128, NQT * K256 * 2 * 128], F8)
            nc.sync.dma_start(out=q8[:, :1024], in_=Q8[:, :1024])
            coff = cst.tile([128, 4], F32)
            nc.sync.dma_start(out=coff[:], in_=COFF[:])
            goff = cst.tile([128, NQT * NC40], F32)
            nc.sync.dma_start(out=goff[:], in_=GOFF[:])
            ident = cst.tile([128, 128], F32)
            nc.sync.dma_start(out=ident[:], in_=IDENT[:])

            # PE warmup on the first fp8 block
            wt = cst.tile([128, 128], F8)
            nc.sync.dma_start(out=wt[:], in_=Q8[:, :128])
            pw = ps.tile([128, 128], F32, tag="p", name="pw")
            nc.tensor.matmul(out=pw[:], lhsT=wt[:], rhs=wt[:],
                             start=True, stop=True)

            # rest of Q8 in the background
            nc.sync.dma_start(out=q8[:, 1024:], in_=Q8[:, 1024:])

            # collective warmup: absorb the ~25us first-AG latency early
            ccw_in = dram.tile([128, 8], F32, tag="ccwi", name="ccw_in")
            ccw_out = dram.tile([NCORES * 128, 8], F32, tag="ccwo",
                                name="ccw_out")
            warm = sm.tile([128, 8], F32, tag="warm", name="warm")
            nc.vector.memset(warm[:], 0.0)
            nc.sync.dma_start(out=ccw_in[:], in_=warm[:])
            nc.gpsimd.collective_compute(
                "AllGather", mybir.AluOpType.bypass,
                replica_groups=[list(range(NCORES))],
                ins=[ccw_in.opt()], outs=[ccw_out.opt()])

            q8v = q8[:].rearrange("p (t k two q) -> p t k two q",
                                  t=NQT, k=K256, two=2)

            def qs8(k, t):
                return q8v[:, t, k]

            # stage-1 candidate (value f32, local idx u16 -> f32) arrays
            valsall = cst.tile([128, NQT * NC40], F32)
            idxu16 = cst.tile([128, NQT * NC40], U16)
            idxall = cst.tile([128, NQT * NC40], F32)

            def stage1_unit(g, t, mts):
                pss = [ps.tile([128, CW], F32, tag="p", name=f"p_{g}_{t}_{ci}")
                       for ci in range(GCH)]
                for k in range(K256):
                    for ci in range(GCH):
                        mv = mts[ci][:].rearrange(
                            "p (k c two) -> p k two c", k=K256, two=2)
                        nc.tensor.matmul(
                            out=pss[ci][:], lhsT=qs8(k, t),
                            rhs=mv[:, k],
                            start=(k == 0), stop=(k == K256 - 1),
                            perf_mode=mybir.MatmulPerfMode.DoubleRow)
                sc = scp.tile([128, GCH * CW], F32, tag="sc")
                for ci in range(GCH):
                    nc.scalar.copy(out=sc[:, ci * CW:(ci + 1) * CW],
                                   in_=pss[ci][:])
                base = t * NC40 + g * 8
                nc.vector.max(out=valsall[:, base:base + 8], in_=sc[:])
                nc.vector.max_index(out=idxu16[:, base:base + 8],
                                    in_max=valsall[:, base:base + 8],
                                    in_values=sc[:])

            def idx_fix(t):
                # local idx u16 -> f32 with group offsets, in one op
                a, b = t * NC40, (t + 1) * NC40
                nc.vector.tensor_tensor(out=idxall[:, a:b],
                                        in0=idxu16[:, a:b], in1=goff[:, a:b],
                                        op=mybir.AluOpType.add)

            def rescore_a(t):
                """Select top-6 and gather their candidate rows."""
                va = valsall[:, t * NC40:(t + 1) * NC40]
                ia = idxall[:, t * NC40:(t + 1) * NC40]
                t8v = sm.tile([128, 8], F32, tag="t8v", name=f"t8v{t}")
                nc.vector.max(out=t8v[:], in_=va)
                eq = tmp.tile([128, T6 * NC40], F32, tag="eq", name=f"eq{t}")
                eq3 = eq[:].rearrange("p (j n) -> p j n", j=T6)
                nc.vector.tensor_tensor(
                    out=eq3, in0=va.unsqueeze(1).to_broadcast([128, T6, NC40]),
                    in1=t8v[:, 0:T6].unsqueeze(2).to_broadcast([128, T6, NC40]),
                    op=mybir.AluOpType.is_equal)
                nc.vector.tensor_tensor(
                    out=eq3, in0=eq3,
                    in1=ia.unsqueeze(1).to_broadcast([128, T6, NC40]),
                    op=mybir.AluOpType.mult)
                i6f = sm.tile([128, T6], F32, tag="i6f", name=f"i6f{t}")
                nc.vector.tensor_reduce(
                    out=i6f[:], in_=eq3,
                    axis=mybir.AxisListType.X, op=mybir.AluOpType.max)
                i6 = sm.tile([128, T6], I32, tag="i6", name=f"i6_{t}")
                nc.scalar.copy(out=i6[:], in_=i6f[:])

                return i6f, i6

            def rescore_a2(t, st):
                # gather bf16 hi||lo rows of the 6 candidates
                i6f, i6 = st
                ghl = gat.tile([128, T6 * 2 * F], BF16, tag="g", name=f"ghl{t}")
                for j in range(T6):
                    nc.gpsimd.indirect_dma_start(
                        out=ghl[:, j * 2 * F:(j + 1) * 2 * F],
                        out_offset=None, in_=MHL[:],
                        in_offset=IndirectOffsetOnAxis(ap=i6[:, j:j + 1],
                                                       axis=0))
                return i6f, ghl

            def rescore_b1(t, st):
                # bf16 Q hi/lo for this tile + ONE XBAR transpose, both on
                # the sync queue (XBAR transposes must serialize)
                i6f, ghl = st
                qht = qh3.tile([128, 2 * KCH * 128], BF16, tag="qht",
                               name=f"qht{t}")
                nc.sync.dma_start(
                    out=qht[:, :KCH * 128],
                    in_=QHL[:, t * KCH * 128:(t + 1) * KCH * 128])
                nc.sync.dma_start(
                    out=qht[:, KCH * 128:],
                    in_=QHL[:, HI_OFF + t * KCH * 128:
                            HI_OFF + (t + 1) * KCH * 128])
                ght = gat.tile([128, T6 * 2 * KCH, 128], BF16, tag="gt",
                               name=f"ght{t}")
                nc.sync.dma_start(out=ght[:], in_=ghl[:], transpose=True)
                return qht, i6f, ght

            def rescore_b2(t, st):
                qht, i6f, ght = st
                gv3 = ght[:].rearrange("p (j h k) q -> p h k j q", h=2, k=KCH)

                # exact bf16x3 rescore, weight-stationary over the 2 halves
                pr = psr.tile([128, 512], F32, tag="pr", name=f"pr{t}")
                i = 0
                for hq, hm in ((0, 0), (0, 1), (1, 0)):
                    for k in range(KCH):
                        base = hq * KCH * 128 + k * 128
                        w = qht[:, base:base + 128]
                        nc.tensor.matmul(
                            out=pr[:], lhsT=w, rhs=gv3[:, hm, k],
                            start=(i == 0), stop=(i == 23))
                        i += 1
                return i6f, pr

            def rescore_b2v(t, st, cc_in):
                i6f, pr = st
                # diagonal extraction from PSUM: s8[q, j] = pr[q, j*128+q];
                # the 4 rescored candidates ARE the AG payload (true top-4
                # sit within the shard's fp8 top-4: measured max rank 3)
                dm = tmp.tile([128, T6 * 128], F32, tag="dm", name=f"dm{t}")
                dm3 = dm[:].rearrange("p (j n) -> p j n", j=T6)
                nc.vector.tensor_tensor(
                    out=dm3,
                    in0=pr[:].rearrange("p (j n) -> p j n", j=T6),
                    in1=ident[:].unsqueeze(1).to_broadcast([128, T6, 128]),
                    op=mybir.AluOpType.mult)
                loc = sm.tile([128, 8], F32, tag="loc", name=f"loc{t}")
                nc.vector.tensor_reduce(
                    out=loc[:, 0:4], in_=dm3,
                    axis=mybir.AxisListType.X, op=mybir.AluOpType.add)
                # global candidate ids
                nc.scalar.activation(out=loc[:, 4:8], in_=i6f[:],
                                     func=mybir.ActivationFunctionType.Identity,
                                     bias=coff[:, 0:1])
                tl = t % 2
                nc.sync.dma_start(out=cc_in[tl * 128:(tl + 1) * 128, :],
                                  in_=loc[:])

            def rescore_b3(t, cc_in):
                # per-pair AllGather of the (value, idx) top-4
                cc_out = dram.tile([NCORES * 256, 8], F32, tag="ccout",
                                   name=f"cc_out{t}")
                nc.gpsimd.collective_compute(
                    "AllGather", mybir.AluOpType.bypass,
                    replica_groups=[list(range(NCORES))],
                    ins=[cc_in.opt()], outs=[cc_out.opt()])
                return cc_out

            def merge_tile(t0, cc_out):
                # merge a PAIR of tiles (t0, t0+1) from one AllGather
                cc_view = cc_out[:].rearrange("(r t q) e -> q t r e",
                                              t=2, q=128)
                cands = fin.tile([128, 2 * NCORES * 8], F32, tag="cands",
                                 name=f"cands{t0}")
                cnd4 = cands[:].rearrange("p (t r e) -> p t r e",
                                          t=2, r=NCORES)
                for tl in range(2):
                    nc.gpsimd.dma_start(out=cnd4[:, tl], in_=cc_view[:, tl])
                c4 = cands[:].rearrange("p (t r e) -> p t r e", t=2, r=NCORES)
                gv = fin.tile([128, 2, 8], F32, tag="gv", name=f"gv{t0}")
                eqf = fin.tile([128, 2 * 4 * 32], F32, tag="feq",
                               name=f"feq{t0}")
                gif = fin.tile([128, 2 * 4], F32, tag="gif", name=f"gif{t0}")
                for tl in range(2):
                    cvv = c4[:, tl, :, 0:4]
                    cvi = c4[:, tl, :, 4:8]
                    nc.vector.max(out=gv[:, tl], in_=cvv)
                    f3 = eqf[:].rearrange("p (t j n) -> p t j n",
                                          t=2, j=4)[:, tl].rearrange(
                        "p j (r e) -> p j r e", r=8)
                    nc.vector.tensor_tensor(
                        out=f3,
                        in0=cvv.unsqueeze(1).to_broadcast([128, 4, 8, 4]),
                        in1=gv[:, tl, 0:4].unsqueeze(2).unsqueeze(3)
                            .to_broadcast([128, 4, 8, 4]),
                        op=mybir.AluOpType.is_equal)
                    nc.vector.tensor_tensor(
                        out=f3, in0=f3,
                        in1=cvi.unsqueeze(1).to_broadcast([128, 4, 8, 4]),
                        op=mybir.AluOpType.mult)
                nc.vector.tensor_reduce(
                    out=gif[:].rearrange("p (t j) -> p t j", t=2),
                    in_=eqf[:].rearrange("p (t j r e) -> p t j r e",
                                         t=2, j=4, r=8),
                    axis=mybir.AxisListType.XY, op=mybir.AluOpType.max)
                gii = fin.tile([128, 2 * 4], I32, tag="gii", name=f"gii{t0}")
                nc.scalar.copy(out=gii[:], in_=gif[:])
                sg = fin.tile([128, 2 * 4 * FSL], F32, tag="sg",
                              name=f"sg{t0}")
                for j in range(8):
                    nc.gpsimd.indirect_dma_start(
                        out=sg[:, j * FSL:(j + 1) * FSL],
                        out_offset=None, in_=SYN[:],
                        in_offset=IndirectOffsetOnAxis(ap=gii[:, j:j + 1],
                                                       axis=0))
                return sg

            def merge_tile2(t0, sg):
                gbuf = fin.tile([128, 2 * FSL], F32, tag="gbuf",
                                name=f"gbuf{t0}")
                nc.vector.tensor_reduce(
                    out=gbuf[:].rearrange("p (t f) -> p t f", t=2),
                    in_=sg[:].rearrange("p (t j f) -> p t f j", t=2, j=4),
                    axis=mybir.AxisListType.X, op=mybir.AluOpType.add)
                nc.sync.dma_start(
                    out=OUT[t0 * 128:(t0 + 2) * 128, :].rearrange(
                        "(t q) f -> q t f", t=2),
                    in_=gbuf[:].rearrange("p (t f) -> p t f", t=2))

            # ---- driver: block-pipelined; stage-2 phases paced by an
            # XBAR-transpose clock model (ns-accurate, strictly
            # serialized) so no engine stream stalls on an unready
            # transpose, and late-dependency ops (PSUM extract, AG,
            # merge) are emitted only once their inputs are surely done ----
            UNIT = 7600.0       # ns per stage-1 unit
            TR = 10000.0        # ns per XBAR transpose
            events = []         # [due_unit, seq, thunk]
            seq = [0]

            def schedule(due, thunk):
                events.append([due, seq[0], thunk])
                seq[0] += 1

            u = [0]
            xbar = [0.0]        # ns model of when the XBAR queue frees up

            def run_due(all_=False):
                events.sort(key=lambda e: (e[0], e[1]))
                while events and (all_ or events[0][0] <= u[0]):
                    events.pop(0)[2]()

            state = {}
            cc_pair = {}

            def queue_stage2(t):
                idx_fix(t)
                state[t] = rescore_a(t)
                if t % 2 == 0:
                    cc_pair[t // 2] = dram.tile([256, 8], F32, tag="ccin",
                                                name=f"cc_in{t}")
                cc_in = cc_pair[t // 2]

                def a2(t=t):
                    state[t] = rescore_a2(t, state[t])

                def b1(t=t):
                    state[t] = rescore_b1(t, state[t])

                def b2(t=t):
                    state[t] = rescore_b2(t, state[t])

                def b2v(t=t, ci=cc_in):
                    rescore_b2v(t, state[t], ci)

                def b3(t=t, ci=cc_in):
                    state[t] = rescore_b3(t, ci)

                def mg(t=t):
                    state[t] = merge_tile(t - 1, state[t])

                def mg2(t=t):
                    merge_tile2(t - 1, state[t])

                now = u[0] * UNIT
                tr_start = max(now + 1.3 * UNIT, xbar[0])
                xbar[0] = tr_start + TR
                done = xbar[0] / UNIT
                schedule(max(u[0] + 0.2, tr_start / UNIT - 1.4), a2)
                schedule(tr_start / UNIT, b1)
                schedule(done + 0.4, b2)
                schedule(done + 1.6, b2v)
                if t % 2 == 1:
                    schedule(done + 2.2, b3)
                    schedule(done + 6.0, mg)
                    schedule(done + 7.6, mg2)

            # flatten (block, group) sequence; chunk loads are emitted one
            # group ahead so they never queue behind stage-2 Act ops
            gseq = []
            t0 = 0
            for b, nt in enumerate(BLOCKS):
                for g in range(NGRP):
                    gseq.append((b, g, range(t0, t0 + nt)))
                t0 += nt

            mts_of = {}

            def emit_loads(gi):
                b, g, _ = gseq[gi]
                mts = []
                for ci in range(GCH):
                    mt = mpool.tile([128, K256 * 2 * CW], F8, tag="mt",
                                    name=f"mt{b}_{g}_{ci}")
                    nc.scalar.dma_start(out=mt[:], in_=M8[g * GCH + ci])
                    mts.append(mt)
                mts_of[gi] = mts

            emit_loads(0)
            for gi, (b, g, tiles) in enumerate(gseq):
                for ti, t in enumerate(tiles):
                    stage1_unit(g, t, mts_of[gi])
                    if ti == 0 and gi + 1 < len(gseq):
                        emit_loads(gi + 1)
                    u[0] += 1
                    run_due()
                    if g == NGRP - 1:
                        queue_stage2(t)
            run_due(all_=True)

    nc.compile()
    return nc


# ---------------- host side ----------------

def _split_bf16(x):
    hi = x.astype(ml_dtypes.bfloat16)
    lo = (x - hi.astype(np.float32)).astype(ml_dtypes.bfloat16)
    return hi, lo


def prepare_inputs(query_seq, matching_set, synth_set):
    """Returns per-core in_maps."""
    q = np.asarray(query_seq, dtype=np.float32)
    m = np.asarray(matching_set, dtype=np.float32)
    syn = np.asarray(synth_set, dtype=np.float32)

    # normalize matching rows with fp64 norms
    norms = np.linalg.norm(m.astype(np.float64), axis=1, keepdims=True)
    mn = (m / norms).astype(np.float32)

    # fp8 Q packed [128, 16*4*2*128]: (k, pair, p, t, q) -> p, (t k pair q)
    qt = np.ascontiguousarray(q.T)                       # [1024, 2048]
    q8 = qt.astype(NPF8).reshape(K256, 2, 128, NQT, 128)
    q8 = q8.transpose(2, 3, 0, 1, 4).reshape(128, NQT * K256 * 2 * 128).copy()

    # bf16 Q hi/lo packed
    qh, ql = _split_bf16(qt)

    def pack_q(a):
        return a.reshape(KCH, 128, NQT, 128).transpose(1, 2, 0, 3).reshape(
            128, KCH * FRM)
    qhl = np.concatenate([pack_q(qh), pack_q(ql)], axis=1).copy()

    # group offsets: slot (t, n) -> 2500 * (n // 8)
    goff1 = np.repeat(np.arange(NGRP, dtype=np.float32) * (GCH * CW), 8)
    goff = np.tile(goff1, NQT)
    goff = np.broadcast_to(goff, (128, NQT * NGRP * 8)).copy()

    ident = np.eye(128, dtype=np.float32)

    in_maps = []
    for core in range(NCORES):
        shard = mn[core * SHARD:(core + 1) * SHARD]      # [12500, 1024]
        mt = np.ascontiguousarray(shard.T)               # [1024, 12500]
        # fp8 M packed [25, 128, 4*500*2]: (k, pair, p, chunk, c) ->
        # chunk, p, (k c pair)
        m8 = mt.astype(NPF8).reshape(K256, 2, 128, NCCH, CW)
        m8 = m8.transpose(3, 2, 0, 4, 1).reshape(NCCH, 128, K256 * 2 * CW).copy()
        # bf16 hi||lo rows for the rescore gather
        mh, ml = _split_bf16(shard)
        mhl = np.concatenate([mh, ml], axis=1)           # [12500, 2048]

        in_maps.append({
            "q8": q8,
            "m8": m8,
            "qhl": qhl,
            "mhl": np.ascontiguousarray(mhl),
            "syn": np.ascontiguousarray(syn[:, core * FSL:(core + 1) * FSL]) * 0.25,
            "coff": np.stack([
                np.full(128, float(core * SHARD), dtype=np.float32),
                np.arange(128, dtype=np.float32),
                np.arange(1, 129, dtype=np.float32),
                np.zeros(128, dtype=np.float32)], axis=1),
            "goff": goff,
            "ident": ident,
        })
    return in_maps


_NC_CACHE = {}


def get_nc():
    if "nc" not in _NC_CACHE:
        _NC_CACHE["nc"] = build()
    return _NC_CACHE["nc"]


def run(query_seq, matching_set, synth_set, topk=4, trace=False):
    assert int(topk) == 4, f"kernel is specialized for topk=4, got {topk}"
    in_maps = prepare_inputs(query_seq, matching_set, synth_set)
    nc = get_nc()
    res = run_bass_kernel_spmd(nc, in_maps, core_ids=list(range(NCORES)),
                               trace=trace)
    out = np.concatenate([res.results[i]["out"] for i in range(NCORES)], axis=1)
    return out.astype(np.float32), res


def kernel(**inputs):
    topk = inputs.get("topk", 4)
    try:
        topk = int(np.asarray(topk))
    except Exception:
        topk = int(topk)
    out, _ = run(inputs["query_seq"], inputs["matching_set"],
                 inputs["synth_set"], topk)
    return out
